# revision 21
# baseline (speedup 1.0000x reference)
"""Trainium2 Bass kernel for the D3CG trainer-loss problem.

Computes, for full inputs:
    loss = sum((eps_theta - noise)**2)
where eps_theta is a 1x1-conv surrogate denoiser applied to
[d_t, cbct_coeffs] built from Haar DWT coefficients of x_0's two channels.

Strategy (pure data parallel over batch, 4 batches per core on 8 cores):
Everything before the square is linear in (x_0, noise) per output pixel, with
per-batch scalar coefficients, so each 64-image-row slab reduces to tensor-
engine contractions against host-precomputed sparse matrices:
  - x path: fp8e4 data at K=256 via a DoubleRow matmul -- the 2x2 Haar
    column-parity pair is packed as the two fp8 k-tiles, so even+odd columns
    contract in a single instruction.  The W-mixed Haar taps are ~0.05 in
    magnitude, so fp8 weight/data error is absolutely tiny here.
  - noise path: bf16 data against Ln = s_omab*W - I in bf16 (the near--1
    diagonal needs bf16 precision; fp8 here costs ~1% loss bias).
PSUM super-tiles are [128, 1024] fp32 (two banks); matmuls fill each
512-column bank half as its own accumulation group.

The square-and-reduce stage is split across the vector and scalar engines
at super-tile granularity:
  - vector (2 passes): tensor_scalar adds the per-partition bias while
    copying PSUM -> SBUF bf16, then scalar_tensor_tensor (r+0)*r with
    accum_out -> sum(r^2) per partition,
  - scalar: activation Square with native bias, accum_out.
A final tensor_reduce + ones-matmul collapses the partials to a scalar;
host sums the 8 per-core values.
"""

import sys

if "/opt/trn_rl_repo" not in sys.path:
    sys.path.insert(0, "/opt/trn_rl_repo")

import ml_dtypes
import numpy as np

import concourse.bass as bass  # noqa: F401
import concourse.mybir as mybir
import concourse.tile as tile
from concourse import bacc
from concourse.bass_utils import run_bass_kernel_spmd

T = 1000
BETA_1 = 1e-4
BETA_T = 0.02

N_CORES = 8
B_TOTAL = 32
B_PER = B_TOTAL // N_CORES  # 4 batches per core
H = 512
Wd = 512
WO = Wd // 2
ROWS_PER_SLAB = 64            # image rows per slab-group (ct + cb -> 128 parts)
N_SLABS = H // ROWS_PER_SLAB  # 8
PAIRS = ROWS_PER_SLAB // 2    # 32 output rows per slab
COLS = N_SLABS * WO           # 2048 free columns per batch (per parity)
NST = COLS // 1024            # 2 PSUM super-tiles [128, 1024] per batch

F32 = mybir.dt.float32
BF16 = mybir.dt.bfloat16
FP8 = mybir.dt.float8e4
NP_BF16 = ml_dtypes.bfloat16
NP_FP8 = ml_dtypes.float8_e4m3

# square-stage engine split per (batch, super-tile): True -> vector (2-pass),
# False -> scalar activation.  3 DVE / 5 ACT balances the engines.
DVE_TILE = [
    [False, True],
    [False, True],
    [False, True],
    [False, False],
]

# Haar 2x2 analysis kernels for [cA, cH, cV, cD] over [[a,b],[c,d]].
_HAAR = 0.5 * np.array(
    [
        [[1.0, 1.0], [1.0, 1.0]],    # cA
        [[1.0, 1.0], [-1.0, -1.0]],  # cH
        [[1.0, -1.0], [1.0, -1.0]],  # cV
        [[1.0, -1.0], [-1.0, 1.0]],  # cD
    ],
    dtype=np.float64,
)


def _schedule():
    betas = np.linspace(BETA_1, BETA_T, T, dtype=np.float64)
    return np.cumprod(1.0 - betas)


def _host_constants(W, b, temb, t):
    """Per-batch lhsT matrices + bias.

    Lx: [128, B, 2, 128] (k, batch, col-parity ktile, m); Ln: [128, B, 128];
    bias: [128, B].
    """
    W = np.asarray(W, dtype=np.float64)
    b = np.asarray(b, dtype=np.float64)
    temb = np.asarray(temb, dtype=np.float64)
    t = np.asarray(t).astype(np.int64)

    alphas_bar = _schedule()
    s_ab = np.sqrt(alphas_bar[t])
    s_omab = np.sqrt(1.0 - alphas_bar[t])

    B = t.shape[0]
    Lx = np.zeros((128, B, 2, 128), dtype=np.float64)
    Ln = np.zeros((128, B, 128), dtype=np.float64)
    bias = np.zeros((128, B), dtype=np.float64)

    for bi in range(B):
        # eps[o] = s_ab * sum_k W[o,k] haar_k(ct)
        #        + sum_k (W[o,4+k] - s_ab W[o,k]) haar_k(cb)
        #        + s_omab * sum_c W[o,c] n_c + b[o] + temb[t,o]
        # r = eps - noise -> noise coeff C[o,c] = s_omab*W[o,c] - delta_oc
        KA = np.einsum("ok,krc->orc", W[:, 0:4], _HAAR) * s_ab[bi]
        KB = np.einsum("ok,krc->orc", W[:, 4:8] - s_ab[bi] * W[:, 0:4], _HAAR)
        C = s_omab[bi] * W[:, 0:4] - np.eye(4)

        for o in range(4):
            for i in range(PAIRS):
                m = o * PAIRS + i
                for r in range(2):
                    # ct rows on slab partitions 0..63, cb rows on 64..127
                    Lx[2 * i + r, bi, 0, m] = KA[o, r, 0]
                    Lx[2 * i + r, bi, 1, m] = KA[o, r, 1]
                    Lx[64 + 2 * i + r, bi, 0, m] = KB[o, r, 0]
                    Lx[64 + 2 * i + r, bi, 1, m] = KB[o, r, 1]
                for c in range(4):
                    Ln[c * PAIRS + i, bi, m] = C[o, c]
            bias[o * PAIRS : (o + 1) * PAIRS, bi] = b[o] + temb[t[bi], o]

    return Lx, Ln, bias


def _shuffle_x0(x0_shard):
    """[B,2,512,512] -> [B, 128, 2, 2048] fp8e4.

    partition p = ch*64 + row-within-slab; free = (parity ktile, slab, outcol).
    """
    B = x0_shard.shape[0]
    v = x0_shard.reshape(B, 2, N_SLABS, ROWS_PER_SLAB, WO, 2)
    # -> [B, ch, row, parity, slab, outcol]
    v = v.transpose(0, 1, 3, 5, 2, 4)
    v = v.reshape(B, 128, 2, 2, COLS // 2)  # split (slab,col) into chunks
    v = v.transpose(0, 3, 1, 2, 4)          # -> [B, chunk, part, parity, col]
    return np.ascontiguousarray(v.astype(NP_FP8))


def _shuffle_nz(nz_shard):
    """[B,4,256,256] -> [B, 128, 2048] bf16; p = c*32 + row-within-slab."""
    B = nz_shard.shape[0]
    v = nz_shard.reshape(B, 4, N_SLABS, PAIRS, WO)
    v = v.transpose(0, 1, 3, 2, 4)
    v = v.reshape(B, 128, 2, COLS // 2)
    v = v.transpose(0, 2, 1, 3)  # -> [B, chunk, part, col]
    return np.ascontiguousarray(v.astype(NP_BF16))


def build_nc(debug=False):
    """Build the per-core Bass program (same program on all 8 cores)."""
    nc = bacc.Bacc("TRN2", target_bir_lowering=False, debug=debug)

    HC = COLS // 2  # 1024 columns per chunk (one super-tile's worth)
    x_d = nc.declare_dram_parameter("x", [B_PER, 2, 128, 2, HC], FP8, isOutput=False)
    nz_d = nc.declare_dram_parameter("nz", [B_PER, 2, 128, HC], BF16, isOutput=False)
    lx_d = nc.declare_dram_parameter("Lx", [128, B_PER, 2, 128], FP8, isOutput=False)
    ln_d = nc.declare_dram_parameter("Ln", [128, B_PER, 128], BF16, isOutput=False)
    bias_d = nc.declare_dram_parameter("bias", [128, B_PER], F32, isOutput=False)
    out_d = nc.declare_dram_parameter("out", [1, 1], F32, isOutput=True)

    NPART = B_PER * NST  # 8 super-tile partials

    with tile.TileContext(nc) as tc:
        with (
            tc.tile_pool(name="consts", bufs=1) as consts,
            tc.tile_pool(name="xp", bufs=8) as xpool,
            tc.tile_pool(name="nzp", bufs=8) as nzpool,
            tc.tile_pool(name="psum", bufs=3, space="PSUM") as psum_pool,
        ):
            lx_t = consts.tile([128, B_PER, 2, 128], FP8, tag="lx")
            ln_t = consts.tile([128, B_PER, 128], BF16, tag="ln")
            bias_t = consts.tile([128, B_PER], F32, tag="bias")
            partials = consts.tile([128, NPART], F32, tag="partials")
            ones = consts.tile([128, 1], F32, tag="ones")
            act_warm = consts.tile([128, 1], F32, tag="act_warm")

            # x DMAs issue immediately on the gpsimd SWDGE queue, one chunk
            # (= one super-tile's data) at a time so the first matmul can
            # start as soon as chunk (0,0) lands; weights, bias and noise
            # issue in parallel on the sync HWDGE queue, batch-0 first.
            xts = []
            for b in range(B_PER):
                for c in range(2):
                    xt = xpool.tile([128, 2, HC], FP8, name=f"xt{b}{c}", tag="xt")
                    nc.gpsimd.dma_start(xt[:], x_d[b, c])
                    xts.append(xt)
            nzts = []
            nc.sync.dma_start(lx_t[:], lx_d[:])
            nzt00 = nzpool.tile([128, HC], BF16, name="nzt00", tag="nzt")
            nc.sync.dma_start(nzt00[:], nz_d[0, 0])
            nzts.append(nzt00)
            nc.sync.dma_start(ln_t[:], ln_d[:])
            nc.sync.dma_start(bias_t[:], bias_d[:])
            for bc in range(1, 2 * B_PER):
                b, c = bc // 2, bc % 2
                nzt = nzpool.tile([128, HC], BF16, name=f"nzt{b}{c}", tag="nzt")
                nc.sync.dma_start(nzt[:], nz_d[b, c])
                nzts.append(nzt)

            # warm the ACT Square table during the DMA ramp
            nc.vector.memset(ones[:], 1.0)
            nc.scalar.activation(
                act_warm[:], ones[:], mybir.ActivationFunctionType.Square
            )

            # warm-up matmuls: stream columns through the idle PE during the
            # DMA ramp so the clock governor starts ramping before real work
            warm_src = consts.tile([128, 512], BF16, tag="warm_src")
            nc.vector.memset(warm_src[:], 1.0)
            warm_ps = psum_pool.tile([1, 512], F32, tag="warm_ps", bufs=1)
            for _ in range(5):
                nc.tensor.matmul(
                    warm_ps[:], warm_src[:, 0:1], warm_src[:],
                    start=True, stop=True,
                )

            for b in range(B_PER):
                for st in range(NST):
                    xt = xts[b * NST + st]
                    nzt = nzts[b * NST + st]
                    ps = psum_pool.tile([128, 1024], F32)
                    for h in range(2):
                        sl = slice(h * 512, (h + 1) * 512)
                        po = slice(h * 512, (h + 1) * 512)
                        # DoubleRow: lhsT [K=128, 2, 128], rhs [K=128, 2, 512]
                        nc.tensor.matmul(
                            ps[:, po],
                            lx_t[:, b, :, :],
                            xt[:, :, sl],
                            start=True,
                            stop=False,
                            perf_mode=mybir.MatmulPerfMode.DoubleRow,
                        )
                        nc.tensor.matmul(
                            ps[:, po], ln_t[:, b, :], nzt[:, sl],
                            start=False, stop=True,
                        )

                    idx = b * NST + st
                    if DVE_TILE[b][st]:
                        scr = consts.tile([128, 1024], BF16, tag="dve_scr", bufs=2)
                        nc.vector.tensor_scalar(
                            out=scr[:],
                            in0=ps[:],
                            scalar1=bias_t[:, b : b + 1],
                            scalar2=None,
                            op0=mybir.AluOpType.add,
                        )
                        scr2 = consts.tile([128, 1024], BF16, tag="dve_scr2", bufs=2)
                        nc.vector.scalar_tensor_tensor(
                            out=scr2[:],
                            in0=scr[:],
                            scalar=0.0,
                            in1=scr[:],
                            op0=mybir.AluOpType.add,
                            op1=mybir.AluOpType.mult,
                            accum_out=partials[:, idx : idx + 1],
                        )
                    else:
                        scr = consts.tile([128, 1024], BF16, tag="act_scr", bufs=2)
                        nc.scalar.activation(
                            scr[:],
                            ps[:],
                            mybir.ActivationFunctionType.Square,
                            bias=bias_t[:, b : b + 1],
                            scale=1.0,
                            accum_out=partials[:, idx : idx + 1],
                        )

            # reduce [128, NPART] partials -> [128,1] -> scalar via ones-matmul
            red = consts.tile([128, 1], F32, tag="red")
            nc.vector.tensor_reduce(
                red[:], partials[:], axis=mybir.AxisListType.X, op=mybir.AluOpType.add
            )
            fin = psum_pool.tile([1, 1], F32, tag="fin", bufs=1)
            nc.tensor.matmul(fin[:], red[:], ones[:], start=True, stop=True)
            out_sb = consts.tile([1, 1], F32, tag="out_sb")
            nc.vector.tensor_copy(out_sb[:], fin[:])
            nc.sync.dma_start(out_d[:], out_sb[:])

    nc.compile()
    return nc


_NC_CACHE = None


def _get_nc():
    global _NC_CACHE
    if _NC_CACHE is None:
        _NC_CACHE = build_nc()
    return _NC_CACHE


def make_in_maps(x_0, noise, W, b, temb, t):
    x_0 = np.asarray(x_0, dtype=np.float32)
    noise = np.asarray(noise, dtype=np.float32)
    Lx, Ln, bias = _host_constants(W, b, temb, t)

    in_maps = []
    for c in range(N_CORES):
        s = slice(c * B_PER, (c + 1) * B_PER)
        in_maps.append(
            {
                "x": _shuffle_x0(x_0[s]),
                "nz": _shuffle_nz(noise[s]),
                "Lx": np.ascontiguousarray(Lx[:, s]).astype(NP_FP8),
                "Ln": np.ascontiguousarray(Ln[:, s]).astype(NP_BF16),
                "bias": np.ascontiguousarray(bias[:, s]).astype(np.float32),
            }
        )
    return in_maps


def kernel(x_0, noise, W, b, temb, t, **_ignored):
    nc = _get_nc()
    in_maps = make_in_maps(x_0, noise, W, b, temb, t)
    res = run_bass_kernel_spmd(nc, in_maps, list(range(N_CORES)))
    total = 0.0
    for c in range(N_CORES):
        total += float(res.results[c]["out"][0, 0])
    return np.float32(total)


# revision 22
# speedup vs baseline: 1.0663x; 1.0663x over previous
"""Trainium2 Bass kernel for the D3CG trainer-loss problem.

Computes, for full inputs:
    loss = sum((eps_theta - noise)**2)
where eps_theta is a 1x1-conv surrogate denoiser applied to
[d_t, cbct_coeffs] built from Haar DWT coefficients of x_0's two channels.

Strategy (pure data parallel over batch, 4 batches per core on 8 cores):
Everything before the square is linear in (x_0, noise) per output pixel, with
per-batch scalar coefficients, so each 64-image-row slab reduces to tensor-
engine contractions against host-precomputed sparse matrices:
  - x path: fp8e4 data at K=256 via a DoubleRow matmul -- the 2x2 Haar
    column-parity pair is packed as the two fp8 k-tiles, so even+odd columns
    contract in a single instruction.  The W-mixed Haar taps are ~0.05 in
    magnitude, so fp8 weight/data error is absolutely tiny here.
  - noise path: bf16 data against Ln = s_omab*W - I in bf16 (the near--1
    diagonal needs bf16 precision; fp8 here costs ~1% loss bias).
PSUM super-tiles are [128, 1024] fp32 (two banks); matmuls fill each
512-column bank half as its own accumulation group.

The square-and-reduce stage is split across the vector and scalar engines
at super-tile granularity:
  - vector (2 passes): tensor_scalar adds the per-partition bias while
    copying PSUM -> SBUF bf16, then scalar_tensor_tensor (r+0)*r with
    accum_out -> sum(r^2) per partition,
  - scalar: activation Square with native bias, accum_out.
A final tensor_reduce + ones-matmul collapses the partials to a scalar;
host sums the 8 per-core values.
"""

import sys

if "/opt/trn_rl_repo" not in sys.path:
    sys.path.insert(0, "/opt/trn_rl_repo")

import ml_dtypes
import numpy as np

import concourse.bass as bass  # noqa: F401
import concourse.mybir as mybir
import concourse.tile as tile
from concourse import bacc
from concourse.bass_utils import run_bass_kernel_spmd

T = 1000
BETA_1 = 1e-4
BETA_T = 0.02

N_CORES = 8
B_TOTAL = 32
B_PER = B_TOTAL // N_CORES  # 4 batches per core
H = 512
Wd = 512
WO = Wd // 2
ROWS_PER_SLAB = 64            # image rows per slab-group (ct + cb -> 128 parts)
N_SLABS = H // ROWS_PER_SLAB  # 8
PAIRS = ROWS_PER_SLAB // 2    # 32 output rows per slab
COLS = N_SLABS * WO           # 2048 free columns per batch (per parity)
NST = COLS // 1024            # 2 PSUM super-tiles [128, 1024] per batch

F32 = mybir.dt.float32
BF16 = mybir.dt.bfloat16
FP8 = mybir.dt.float8e4
NP_BF16 = ml_dtypes.bfloat16
NP_FP8 = ml_dtypes.float8_e4m3

# square-stage engine split per (batch, super-tile): True -> vector (2-pass),
# False -> scalar activation.  3 DVE / 5 ACT balances the engines.
DVE_TILE = [
    [False, True],
    [False, True],
    [False, True],
    [False, False],
]

# Haar 2x2 analysis kernels for [cA, cH, cV, cD] over [[a,b],[c,d]].
_HAAR = 0.5 * np.array(
    [
        [[1.0, 1.0], [1.0, 1.0]],    # cA
        [[1.0, 1.0], [-1.0, -1.0]],  # cH
        [[1.0, -1.0], [1.0, -1.0]],  # cV
        [[1.0, -1.0], [-1.0, 1.0]],  # cD
    ],
    dtype=np.float64,
)


def _schedule():
    betas = np.linspace(BETA_1, BETA_T, T, dtype=np.float64)
    return np.cumprod(1.0 - betas)


def _host_constants(W, b, temb, t):
    """Per-batch lhsT matrices + bias.

    Lx: [128, B, 2, 128] (k, batch, col-parity ktile, m); Ln: [128, B, 128];
    bias: [128, B].
    """
    W = np.asarray(W, dtype=np.float64)
    b = np.asarray(b, dtype=np.float64)
    temb = np.asarray(temb, dtype=np.float64)
    t = np.asarray(t).astype(np.int64)

    alphas_bar = _schedule()
    s_ab = np.sqrt(alphas_bar[t])
    s_omab = np.sqrt(1.0 - alphas_bar[t])

    B = t.shape[0]
    Lx = np.zeros((128, B, 2, 128), dtype=np.float64)
    Ln = np.zeros((128, B, 128), dtype=np.float64)
    bias = np.zeros((128, B), dtype=np.float64)

    for bi in range(B):
        # eps[o] = s_ab * sum_k W[o,k] haar_k(ct)
        #        + sum_k (W[o,4+k] - s_ab W[o,k]) haar_k(cb)
        #        + s_omab * sum_c W[o,c] n_c + b[o] + temb[t,o]
        # r = eps - noise -> noise coeff C[o,c] = s_omab*W[o,c] - delta_oc
        KA = np.einsum("ok,krc->orc", W[:, 0:4], _HAAR) * s_ab[bi]
        KB = np.einsum("ok,krc->orc", W[:, 4:8] - s_ab[bi] * W[:, 0:4], _HAAR)
        C = s_omab[bi] * W[:, 0:4] - np.eye(4)

        for o in range(4):
            for i in range(PAIRS):
                m = o * PAIRS + i
                for r in range(2):
                    # ct rows on slab partitions 0..63, cb rows on 64..127
                    Lx[2 * i + r, bi, 0, m] = KA[o, r, 0]
                    Lx[2 * i + r, bi, 1, m] = KA[o, r, 1]
                    Lx[64 + 2 * i + r, bi, 0, m] = KB[o, r, 0]
                    Lx[64 + 2 * i + r, bi, 1, m] = KB[o, r, 1]
                for c in range(4):
                    Ln[c * PAIRS + i, bi, m] = C[o, c]
            bias[o * PAIRS : (o + 1) * PAIRS, bi] = b[o] + temb[t[bi], o]

    return Lx, Ln, bias


def _shuffle_x0(x0_shard):
    """[B,2,512,512] -> [B, 128, 2, 2048] fp8e4.

    partition p = ch*64 + row-within-slab; free = (parity ktile, slab, outcol).
    """
    B = x0_shard.shape[0]
    v = x0_shard.reshape(B, 2, N_SLABS, ROWS_PER_SLAB, WO, 2)
    # -> [B, ch, row, parity, slab, outcol]
    v = v.transpose(0, 1, 3, 5, 2, 4)
    return np.ascontiguousarray(v.reshape(B, 128, 2, COLS).astype(NP_FP8))


def _shuffle_nz(nz_shard):
    """[B,4,256,256] -> [B, 128, 2048] bf16; p = c*32 + row-within-slab."""
    B = nz_shard.shape[0]
    v = nz_shard.reshape(B, 4, N_SLABS, PAIRS, WO)
    v = v.transpose(0, 1, 3, 2, 4)
    return np.ascontiguousarray(v.reshape(B, 128, COLS).astype(NP_BF16))


def build_nc(debug=False):
    """Build the per-core Bass program (same program on all 8 cores)."""
    nc = bacc.Bacc("TRN2", target_bir_lowering=False, debug=debug)

    x_d = nc.declare_dram_parameter("x", [B_PER, 128, 2, COLS], FP8, isOutput=False)
    nz_d = nc.declare_dram_parameter("nz", [B_PER, 128, COLS], BF16, isOutput=False)
    lx_d = nc.declare_dram_parameter("Lx", [128, B_PER, 2, 128], FP8, isOutput=False)
    ln_d = nc.declare_dram_parameter("Ln", [128, B_PER, 128], BF16, isOutput=False)
    bias_d = nc.declare_dram_parameter("bias", [128, B_PER], F32, isOutput=False)
    out_d = nc.declare_dram_parameter("out", [1, 1], F32, isOutput=True)

    NPART = B_PER * NST  # 8 super-tile partials

    with tile.TileContext(nc) as tc:
        with (
            tc.tile_pool(name="consts", bufs=1) as consts,
            tc.tile_pool(name="xp", bufs=4) as xpool,
            tc.tile_pool(name="nzp", bufs=4) as nzpool,
            tc.tile_pool(name="psum", bufs=3, space="PSUM") as psum_pool,
        ):
            lx_t = consts.tile([128, B_PER, 2, 128], FP8, tag="lx")
            ln_t = consts.tile([128, B_PER, 128], BF16, tag="ln")
            bias_t = consts.tile([128, B_PER], F32, tag="bias")
            partials = consts.tile([128, NPART], F32, tag="partials")
            ones = consts.tile([128, 1], F32, tag="ones")
            act_warm = consts.tile([128, 1], F32, tag="act_warm")

            # x DMAs issue immediately on the gpsimd SWDGE queue; everything
            # else (weights, bias, noise) issues in parallel on the sync
            # HWDGE queue, ordered so batch-0 dependencies land first.
            xts = []
            for b in range(B_PER):
                xt = xpool.tile([128, 2, COLS], FP8, name=f"xt{b}", tag="xt")
                nc.gpsimd.dma_start(xt[:], x_d[b])
                xts.append(xt)
            nzts = []
            nc.sync.dma_start(lx_t[:], lx_d[:])
            nzt0 = nzpool.tile([128, COLS], BF16, name="nzt0", tag="nzt")
            nc.sync.dma_start(nzt0[:], nz_d[0])
            nzts.append(nzt0)
            nc.sync.dma_start(ln_t[:], ln_d[:])
            nc.sync.dma_start(bias_t[:], bias_d[:])
            for b in range(1, B_PER):
                nzt = nzpool.tile([128, COLS], BF16, name=f"nzt{b}", tag="nzt")
                nc.sync.dma_start(nzt[:], nz_d[b])
                nzts.append(nzt)

            # warm the ACT Square table during the DMA ramp
            nc.vector.memset(ones[:], 1.0)
            nc.scalar.activation(
                act_warm[:], ones[:], mybir.ActivationFunctionType.Square
            )

            for b in range(B_PER):
                xt = xts[b]
                nzt = nzts[b]
                for st in range(NST):
                    ps = psum_pool.tile([128, 1024], F32)
                    for h in range(2):
                        t = st * 2 + h
                        sl = slice(t * 512, (t + 1) * 512)
                        po = slice(h * 512, (h + 1) * 512)
                        # DoubleRow: lhsT [K=128, 2, 128], rhs [K=128, 2, 512]
                        nc.tensor.matmul(
                            ps[:, po],
                            lx_t[:, b, :, :],
                            xt[:, :, sl],
                            start=True,
                            stop=False,
                            perf_mode=mybir.MatmulPerfMode.DoubleRow,
                        )
                        nc.tensor.matmul(
                            ps[:, po], ln_t[:, b, :], nzt[:, sl],
                            start=False, stop=True,
                        )

                    idx = b * NST + st
                    if DVE_TILE[b][st]:
                        scr = consts.tile([128, 1024], BF16, tag="dve_scr", bufs=2)
                        nc.vector.tensor_scalar(
                            out=scr[:],
                            in0=ps[:],
                            scalar1=bias_t[:, b : b + 1],
                            scalar2=None,
                            op0=mybir.AluOpType.add,
                        )
                        scr2 = consts.tile([128, 1024], BF16, tag="dve_scr2", bufs=2)
                        nc.vector.scalar_tensor_tensor(
                            out=scr2[:],
                            in0=scr[:],
                            scalar=0.0,
                            in1=scr[:],
                            op0=mybir.AluOpType.add,
                            op1=mybir.AluOpType.mult,
                            accum_out=partials[:, idx : idx + 1],
                        )
                    else:
                        scr = consts.tile([128, 1024], BF16, tag="act_scr", bufs=2)
                        nc.scalar.activation(
                            scr[:],
                            ps[:],
                            mybir.ActivationFunctionType.Square,
                            bias=bias_t[:, b : b + 1],
                            scale=1.0,
                            accum_out=partials[:, idx : idx + 1],
                        )

            # reduce [128, NPART] partials -> [128,1] -> scalar via ones-matmul
            red = consts.tile([128, 1], F32, tag="red")
            nc.vector.tensor_reduce(
                red[:], partials[:], axis=mybir.AxisListType.X, op=mybir.AluOpType.add
            )
            fin = psum_pool.tile([1, 1], F32, tag="fin", bufs=1)
            nc.tensor.matmul(fin[:], red[:], ones[:], start=True, stop=True)
            out_sb = consts.tile([1, 1], F32, tag="out_sb")
            nc.vector.tensor_copy(out_sb[:], fin[:])
            nc.sync.dma_start(out_d[:], out_sb[:])

    nc.compile()
    return nc


_NC_CACHE = None


def _get_nc():
    global _NC_CACHE
    if _NC_CACHE is None:
        _NC_CACHE = build_nc()
    return _NC_CACHE


def make_in_maps(x_0, noise, W, b, temb, t):
    x_0 = np.asarray(x_0, dtype=np.float32)
    noise = np.asarray(noise, dtype=np.float32)
    Lx, Ln, bias = _host_constants(W, b, temb, t)

    in_maps = []
    for c in range(N_CORES):
        s = slice(c * B_PER, (c + 1) * B_PER)
        in_maps.append(
            {
                "x": _shuffle_x0(x_0[s]),
                "nz": _shuffle_nz(noise[s]),
                "Lx": np.ascontiguousarray(Lx[:, s]).astype(NP_FP8),
                "Ln": np.ascontiguousarray(Ln[:, s]).astype(NP_BF16),
                "bias": np.ascontiguousarray(bias[:, s]).astype(np.float32),
            }
        )
    return in_maps


def kernel(x_0, noise, W, b, temb, t, **_ignored):
    nc = _get_nc()
    in_maps = make_in_maps(x_0, noise, W, b, temb, t)
    res = run_bass_kernel_spmd(nc, in_maps, list(range(N_CORES)))
    total = 0.0
    for c in range(N_CORES):
        total += float(res.results[c]["out"][0, 0])
    return np.float32(total)


# revision 23
# speedup vs baseline: 1.1007x; 1.0322x over previous
"""Trainium2 Bass kernel for the D3CG trainer-loss problem.

Computes, for full inputs:
    loss = sum((eps_theta - noise)**2)
where eps_theta is a 1x1-conv surrogate denoiser applied to
[d_t, cbct_coeffs] built from Haar DWT coefficients of x_0's two channels.

Strategy (pure data parallel over batch, 4 batches per core on 8 cores):
Everything before the square is linear in (x_0, noise) per output pixel, with
per-batch scalar coefficients, so each 64-image-row slab reduces to tensor-
engine contractions against host-precomputed sparse matrices:
  - x path: fp8e4 data at K=256 via a DoubleRow matmul -- the 2x2 Haar
    column-parity pair is packed as the two fp8 k-tiles, so even+odd columns
    contract in a single instruction.  The W-mixed Haar taps are ~0.05 in
    magnitude, so fp8 weight/data error is absolutely tiny here.
  - noise path: bf16 data against Ln = s_omab*W - I in bf16 (the near--1
    diagonal needs bf16 precision; fp8 here costs ~1% loss bias).
PSUM super-tiles are [128, 1024] fp32 (two banks); matmuls fill each
512-column bank half as its own accumulation group.

The square-and-reduce stage is split across the vector and scalar engines
at super-tile granularity:
  - vector (2 passes): tensor_scalar adds the per-partition bias while
    copying PSUM -> SBUF bf16, then scalar_tensor_tensor (r+0)*r with
    accum_out -> sum(r^2) per partition,
  - scalar: activation Square with native bias, accum_out.
A final tensor_reduce + ones-matmul collapses the partials to a scalar;
host sums the 8 per-core values.
"""

import sys

if "/opt/trn_rl_repo" not in sys.path:
    sys.path.insert(0, "/opt/trn_rl_repo")

import ml_dtypes
import numpy as np

import concourse.bass as bass  # noqa: F401
import concourse.mybir as mybir
import concourse.tile as tile
from concourse import bacc
from concourse.bass_utils import run_bass_kernel_spmd

T = 1000
BETA_1 = 1e-4
BETA_T = 0.02

N_CORES = 8
B_TOTAL = 32
B_PER = B_TOTAL // N_CORES  # 4 batches per core
H = 512
Wd = 512
WO = Wd // 2
ROWS_PER_SLAB = 64            # image rows per slab-group (ct + cb -> 128 parts)
N_SLABS = H // ROWS_PER_SLAB  # 8
PAIRS = ROWS_PER_SLAB // 2    # 32 output rows per slab
COLS = N_SLABS * WO           # 2048 free columns per batch (per parity)
NST = COLS // 1024            # 2 PSUM super-tiles [128, 1024] per batch

F32 = mybir.dt.float32
BF16 = mybir.dt.bfloat16
FP8 = mybir.dt.float8e4
NP_BF16 = ml_dtypes.bfloat16
NP_FP8 = ml_dtypes.float8_e4m3

# square-stage engine split per (batch, super-tile): True -> vector (2-pass),
# False -> scalar activation.  3 DVE / 5 ACT balances the engines.
DVE_TILE = [
    [False, True],
    [False, True],
    [False, True],
    [False, False],
]

# Haar 2x2 analysis kernels for [cA, cH, cV, cD] over [[a,b],[c,d]].
_HAAR = 0.5 * np.array(
    [
        [[1.0, 1.0], [1.0, 1.0]],    # cA
        [[1.0, 1.0], [-1.0, -1.0]],  # cH
        [[1.0, -1.0], [1.0, -1.0]],  # cV
        [[1.0, -1.0], [-1.0, 1.0]],  # cD
    ],
    dtype=np.float64,
)


def _schedule():
    betas = np.linspace(BETA_1, BETA_T, T, dtype=np.float64)
    return np.cumprod(1.0 - betas)


def _host_constants(W, b, temb, t):
    """Per-batch lhsT matrices + bias.

    Lx: [128, B, 2, 128] (k, batch, col-parity ktile, m); Ln: [128, B, 128];
    bias: [128, B].
    """
    W = np.asarray(W, dtype=np.float64)
    b = np.asarray(b, dtype=np.float64)
    temb = np.asarray(temb, dtype=np.float64)
    t = np.asarray(t).astype(np.int64)

    alphas_bar = _schedule()
    s_ab = np.sqrt(alphas_bar[t])
    s_omab = np.sqrt(1.0 - alphas_bar[t])

    B = t.shape[0]
    Lx = np.zeros((128, B, 2, 128), dtype=np.float64)
    Ln = np.zeros((128, B, 128), dtype=np.float64)
    bias = np.zeros((128, B), dtype=np.float64)

    for bi in range(B):
        # eps[o] = s_ab * sum_k W[o,k] haar_k(ct)
        #        + sum_k (W[o,4+k] - s_ab W[o,k]) haar_k(cb)
        #        + s_omab * sum_c W[o,c] n_c + b[o] + temb[t,o]
        # r = eps - noise -> noise coeff C[o,c] = s_omab*W[o,c] - delta_oc
        KA = np.einsum("ok,krc->orc", W[:, 0:4], _HAAR) * s_ab[bi]
        KB = np.einsum("ok,krc->orc", W[:, 4:8] - s_ab[bi] * W[:, 0:4], _HAAR)
        C = s_omab[bi] * W[:, 0:4] - np.eye(4)

        for o in range(4):
            for i in range(PAIRS):
                m = o * PAIRS + i
                for r in range(2):
                    # ct rows on slab partitions 0..63, cb rows on 64..127
                    Lx[2 * i + r, bi, 0, m] = KA[o, r, 0]
                    Lx[2 * i + r, bi, 1, m] = KA[o, r, 1]
                    Lx[64 + 2 * i + r, bi, 0, m] = KB[o, r, 0]
                    Lx[64 + 2 * i + r, bi, 1, m] = KB[o, r, 1]
                for c in range(4):
                    Ln[c * PAIRS + i, bi, m] = C[o, c]
            bias[o * PAIRS : (o + 1) * PAIRS, bi] = b[o] + temb[t[bi], o]

    return Lx, Ln, bias


def _shuffle_x0(x0_shard):
    """[B,2,512,512] -> [B, 128, 2, 2048] fp8e4.

    partition p = ch*64 + row-within-slab; free = (parity ktile, slab, outcol).
    """
    B = x0_shard.shape[0]
    v = x0_shard.reshape(B, 2, N_SLABS, ROWS_PER_SLAB, WO, 2)
    # -> [B, ch, row, parity, slab, outcol]
    v = v.transpose(0, 1, 3, 5, 2, 4)
    return np.ascontiguousarray(v.reshape(B, 128, 2, COLS).astype(NP_FP8))


def _shuffle_nz(nz_shard):
    """[B,4,256,256] -> [B, 128, 2048] bf16; p = c*32 + row-within-slab."""
    B = nz_shard.shape[0]
    v = nz_shard.reshape(B, 4, N_SLABS, PAIRS, WO)
    v = v.transpose(0, 1, 3, 2, 4)
    return np.ascontiguousarray(v.reshape(B, 128, COLS).astype(NP_BF16))


def build_nc(debug=False):
    """Build the per-core Bass program (same program on all 8 cores)."""
    nc = bacc.Bacc("TRN2", target_bir_lowering=False, debug=debug)

    x_d = nc.declare_dram_parameter("x", [B_PER, 128, 2, COLS], FP8, isOutput=False)
    nz_d = nc.declare_dram_parameter("nz", [B_PER, 128, COLS], BF16, isOutput=False)
    lx_d = nc.declare_dram_parameter("Lx", [128, B_PER, 2, 128], FP8, isOutput=False)
    ln_d = nc.declare_dram_parameter("Ln", [128, B_PER, 128], BF16, isOutput=False)
    bias_d = nc.declare_dram_parameter("bias", [128, B_PER], F32, isOutput=False)
    out_d = nc.declare_dram_parameter("out", [1, 1], F32, isOutput=True)

    NPART = B_PER * NST  # 8 super-tile partials

    with tile.TileContext(nc) as tc:
        with (
            tc.tile_pool(name="consts", bufs=1) as consts,
            tc.tile_pool(name="xp", bufs=4) as xpool,
            tc.tile_pool(name="nzp", bufs=4) as nzpool,
            tc.tile_pool(name="psum", bufs=3, space="PSUM") as psum_pool,
        ):
            lx_t = consts.tile([128, B_PER, 2, 128], FP8, tag="lx")
            ln_t = consts.tile([128, B_PER, 128], BF16, tag="ln")
            bias_t = consts.tile([128, B_PER], F32, tag="bias")
            partials = consts.tile([128, NPART], F32, tag="partials")
            ones = consts.tile([128, 1], F32, tag="ones")
            act_warm = consts.tile([128, 1], F32, tag="act_warm")

            # x DMAs issue immediately on the gpsimd SWDGE queue; everything
            # else (weights, bias, noise) issues in parallel on the sync
            # HWDGE queue, ordered so batch-0 dependencies land first.
            xts = []
            for b in range(B_PER):
                xt = xpool.tile([128, 2, COLS], FP8, name=f"xt{b}", tag="xt")
                nc.gpsimd.dma_start(xt[:], x_d[b])
                xts.append(xt)
            nzts = []
            nc.sync.dma_start(lx_t[:], lx_d[:])
            nzt0 = nzpool.tile([128, COLS], BF16, name="nzt0", tag="nzt")
            nc.sync.dma_start(nzt0[:], nz_d[0])
            nzts.append(nzt0)
            nc.sync.dma_start(ln_t[:], ln_d[:])
            nc.sync.dma_start(bias_t[:], bias_d[:])
            for b in range(1, B_PER):
                nzt = nzpool.tile([128, COLS], BF16, name=f"nzt{b}", tag="nzt")
                nc.sync.dma_start(nzt[:], nz_d[b])
                nzts.append(nzt)

            # warm the ACT Square table during the DMA ramp
            nc.vector.memset(ones[:], 1.0)
            nc.scalar.activation(
                act_warm[:], ones[:], mybir.ActivationFunctionType.Square
            )

            # warm-up matmuls: stream columns through the idle PE during the
            # DMA ramp so the clock governor starts ramping before real work
            warm_src = consts.tile([128, 512], BF16, tag="warm_src")
            nc.vector.memset(warm_src[:], 1.0)
            warm_ps = psum_pool.tile([1, 512], F32, tag="warm_ps", bufs=1)
            for _ in range(6):
                nc.tensor.matmul(
                    warm_ps[:], warm_src[:, 0:1], warm_src[:],
                    start=True, stop=True,
                )

            for b in range(B_PER):
                xt = xts[b]
                nzt = nzts[b]
                for st in range(NST):
                    ps = psum_pool.tile([128, 1024], F32)
                    for h in range(2):
                        t = st * 2 + h
                        sl = slice(t * 512, (t + 1) * 512)
                        po = slice(h * 512, (h + 1) * 512)
                        # DoubleRow: lhsT [K=128, 2, 128], rhs [K=128, 2, 512]
                        nc.tensor.matmul(
                            ps[:, po],
                            lx_t[:, b, :, :],
                            xt[:, :, sl],
                            start=True,
                            stop=False,
                            perf_mode=mybir.MatmulPerfMode.DoubleRow,
                        )
                        nc.tensor.matmul(
                            ps[:, po], ln_t[:, b, :], nzt[:, sl],
                            start=False, stop=True,
                        )

                    idx = b * NST + st
                    if DVE_TILE[b][st]:
                        scr = consts.tile([128, 1024], BF16, tag="dve_scr", bufs=2)
                        nc.vector.tensor_scalar(
                            out=scr[:],
                            in0=ps[:],
                            scalar1=bias_t[:, b : b + 1],
                            scalar2=None,
                            op0=mybir.AluOpType.add,
                        )
                        scr2 = consts.tile([128, 1024], BF16, tag="dve_scr2", bufs=2)
                        nc.vector.scalar_tensor_tensor(
                            out=scr2[:],
                            in0=scr[:],
                            scalar=0.0,
                            in1=scr[:],
                            op0=mybir.AluOpType.add,
                            op1=mybir.AluOpType.mult,
                            accum_out=partials[:, idx : idx + 1],
                        )
                    else:
                        scr = consts.tile([128, 1024], BF16, tag="act_scr", bufs=2)
                        nc.scalar.activation(
                            scr[:],
                            ps[:],
                            mybir.ActivationFunctionType.Square,
                            bias=bias_t[:, b : b + 1],
                            scale=1.0,
                            accum_out=partials[:, idx : idx + 1],
                        )

            # reduce [128, NPART] partials -> [128,1] -> scalar via ones-matmul
            red = consts.tile([128, 1], F32, tag="red")
            nc.vector.tensor_reduce(
                red[:], partials[:], axis=mybir.AxisListType.X, op=mybir.AluOpType.add
            )
            fin = psum_pool.tile([1, 1], F32, tag="fin", bufs=1)
            nc.tensor.matmul(fin[:], red[:], ones[:], start=True, stop=True)
            out_sb = consts.tile([1, 1], F32, tag="out_sb")
            nc.vector.tensor_copy(out_sb[:], fin[:])
            nc.sync.dma_start(out_d[:], out_sb[:])

    nc.compile()
    return nc


_NC_CACHE = None


def _get_nc():
    global _NC_CACHE
    if _NC_CACHE is None:
        _NC_CACHE = build_nc()
    return _NC_CACHE


def make_in_maps(x_0, noise, W, b, temb, t):
    x_0 = np.asarray(x_0, dtype=np.float32)
    noise = np.asarray(noise, dtype=np.float32)
    Lx, Ln, bias = _host_constants(W, b, temb, t)

    in_maps = []
    for c in range(N_CORES):
        s = slice(c * B_PER, (c + 1) * B_PER)
        in_maps.append(
            {
                "x": _shuffle_x0(x_0[s]),
                "nz": _shuffle_nz(noise[s]),
                "Lx": np.ascontiguousarray(Lx[:, s]).astype(NP_FP8),
                "Ln": np.ascontiguousarray(Ln[:, s]).astype(NP_BF16),
                "bias": np.ascontiguousarray(bias[:, s]).astype(np.float32),
            }
        )
    return in_maps


def kernel(x_0, noise, W, b, temb, t, **_ignored):
    nc = _get_nc()
    in_maps = make_in_maps(x_0, noise, W, b, temb, t)
    res = run_bass_kernel_spmd(nc, in_maps, list(range(N_CORES)))
    total = 0.0
    for c in range(N_CORES):
        total += float(res.results[c]["out"][0, 0])
    return np.float32(total)
